# revision 18
# baseline (speedup 1.0000x reference)
"""Masked attention kernel for Trainium2, 8 NeuronCores.

Problem: q,k,v [32,1024,64] f32, mask [32,1024,1024] bool (True -> -inf),
out = softmax(q@k^T * D^-0.5 masked) @ v.

Sharding: batch*head dim (32) split across 8 cores, 4 heads/core.

v3 design notes (from HW power-monitor analysis): the PE/DVE/Pool clock
domain duty-cycles between 1.2 and 2.4 GHz (HAM '0'/'1' windows in the
profile) while ACT runs at a fixed clock, and a matmul costs one clock per
output column regardless of dtype (fp8 DoubleRow gives no column savings).
So the design minimizes PE columns (QK + AV + tail transposes only, no PE
masking) and keeps ACT exp-only (32 x ~1us = the fixed-clock critical
stream):
  - exp on ACT from the score PSUM (scale=0.125), bf16 out.
  - masks applied post-exp: DVE tensor_mul with bf16 keep {1,0} masks
    (2-byte operands hit the 2x_1p DVE mode: ~0.5us/tile) for 5 tiles per
    head, Pool (gpsimd) with fp8 keep masks for the last 3 (Pool cannot
    read PSUM but p lives in SBUF; fp8 halves its DMA bytes).
  - AV with v_aug [128,80] bf16 stationary; ones-row carries denominators.
  - tail: DVE cast, PE transpose (identity65), DVE recip+broadcast-mul.
Startup: no DMA hoisting (hoisted SP DMAs extend the preamble barrier and
delay every engine's body by their issue time); instead the first triggers
are spread across SP/DVE queues at body start, a gpsimd memset feeds an
immediate ACT exp to load the Exp table during the DMA wait, and a short
chain of junk PE matmuls covers the pipeline fill until real data lands.
"""

import os
import sys

import numpy as np

for _p in ("/opt/trn_rl_repo", "/opt/pypackages"):
    if _p not in sys.path and os.path.isdir(_p):
        sys.path.append(_p)

import ml_dtypes  # noqa: E402

import concourse.bass as bass  # noqa: E402
import concourse.tile as tile  # noqa: E402
from concourse import mybir  # noqa: E402
from concourse.bass_utils import run_bass_kernel_spmd  # noqa: E402

BH, S, D = 32, 1024, 64
NCORES = 8
HPC = BH // NCORES  # heads per core
NT = S // 128  # 8 tiles of 128 along s/t
FP8 = ml_dtypes.float8_e4m3fn
F32 = mybir.dt.float32
BF16 = mybir.dt.bfloat16
DT8 = mybir.dt.float8e4

NDV = 5  # tiles/head masked on DVE (bf16 keep); the rest on Pool (fp8)
NPO = NT - NDV
WARM_N = 0  # qkt[0] is hoisted pre-barrier; no PE warmup needed


def _pool_tiles(h):
    """Pool-masked tiles per head. Pool is slow (~2.3us/tile); for the last
    head its masks would gate the final AV chunks, so give Pool the
    earliest-consumed tiles there instead of the last ones."""
    return (0, 1, 2) if h == HPC - 1 else (NT - NPO, NT - NPO + 1, NT - NPO + 2)


def _build_program():
    nc = bass.Bass(
        "TRN2",
        target_bir_lowering=False,
        debug=False,
        num_devices=NCORES,
    )
    # qkt[h]: [kT tiles (NT*128) | qT (S)], rows duplicated into 64-127 so
    # consecutive kT tiles alternate PE row-quadrants (no stationary-switch
    # drain bubble between QK tiles)
    qkt = nc.dram_tensor("qkt", [HPC, 128, 2 * S], BF16, kind="ExternalInput").ap()
    vaug = nc.dram_tensor("vaug", [HPC, 128, NT * 80], BF16, kind="ExternalInput").ap()
    mdv = nc.dram_tensor("mdv", [HPC, 128, NDV * S], BF16, kind="ExternalInput").ap()
    mpo = nc.dram_tensor("mpo", [HPC, 128, NPO * S], DT8, kind="ExternalInput").ap()
    outp = nc.dram_tensor("outp", [HPC, 128, NT * D], BF16, kind="ExternalOutput").ap()

    with tile.TileContext(nc) as tc:
        with (
            tc.tile_pool(name="const", bufs=1) as const_pool,
            tc.tile_pool(name="qk", bufs=HPC) as qk_pool,
            tc.tile_pool(name="v", bufs=HPC) as v_pool,
            tc.tile_pool(name="mdv", bufs=HPC) as mdv_pool,
            tc.tile_pool(name="mpo", bufs=HPC) as mpo_pool,
            tc.tile_pool(name="p", bufs=3) as p_pool,
            tc.tile_pool(name="ot", bufs=5) as ot_pool,
            tc.tile_pool(name="fin", bufs=2) as fin_pool,
            tc.tile_pool(name="spsum", bufs=3, space="PSUM") as s_pool,
            tc.tile_pool(name="opsum", bufs=2, space="PSUM") as o_pool,
        ):
            warm_sb = const_pool.tile([128, 256], BF16, tag="warm")

            # ACT warmup: memset (gpsimd, write-only) -> exp loads the Exp
            # activation table immediately, overlapping the input DMA wait.
            nc.gpsimd.memset(warm_sb[:], 0.0)
            warm_out = const_pool.tile([1, 1], F32, tag="warmo")
            nc.scalar.activation(
                out=warm_out[:],
                in_=warm_sb[0:1, 0:1],
                func=mybir.ActivationFunctionType.Exp,
            )

            qk_tiles, v_tiles, mdv_tiles, mpo_tiles = [], [], [], []
            for h in range(HPC):
                qk_tiles.append(qk_pool.tile([128, 2 * S], BF16, name="qk_sb"))
                v_tiles.append(v_pool.tile([128, NT * 80], BF16, name="v_sb"))
                mdv_tiles.append(mdv_pool.tile([128, NDV * S], BF16, name="mdv_sb"))
                mpo_tiles.append(mpo_pool.tile([128, NPO * S], DT8, name="mpo_sb"))

            def load_qk(h, eng=None):
                (eng or nc.sync).dma_start(qk_tiles[h][:], qkt[h])

            def load_v(h, eng=None):
                (eng or nc.sync).dma_start(v_tiles[h][:], vaug[h])

            def load_mdv(h, lo, hi, eng=None):
                (eng or nc.sync).dma_start(
                    mdv_tiles[h][:, lo * S : hi * S], mdv[h][:, lo * S : hi * S]
                )

            def load_mpo(h, eng=None):
                (eng or nc.sync).dma_start(mpo_tiles[h][:], mpo[h])

            # First-needed transfers issue in parallel from idle queues at
            # body start; the rest stream on SP.
            load_qk(0)  # SP: first QK dependency (hoisted into the preamble)
            load_mdv(0, 0, 1, eng=nc.scalar)  # ACT queue is idle this early
            load_v(0)
            load_mdv(0, 1, 3, eng=nc.scalar)
            load_mdv(0, 3, NDV)
            load_mpo(0, eng=nc.gpsimd)
            load_qk(1)
            load_mdv(1, 0, NDV)
            load_v(1)
            load_mpo(1)
            load_qk(2)
            load_mdv(2, 0, NDV)
            load_v(2)
            load_mpo(2)
            load_qk(3)
            load_mdv(3, 0, NDV)
            load_v(3)
            load_mpo(3)

            # PE warmup: junk matmuls (dep only on the memset) keep the PE
            # busy from the barrier until real data lands (pipeline fill).
            # Target an o_pool buffer (same shape/tag as the AV accumulators
            # so the pool ring stays 2 banks): the first real o_pool use is
            # head-0's AV, well after the warmups drain, so they never
            # interlock with the QK score banks.
            warm_ps = o_pool.tile([80, 512], F32, tag="ops") if WARM_N else None
            for i in range(WARM_N):
                nc.tensor.matmul(
                    out=warm_ps[:, (i % 8) * 64 : (i % 8) * 64 + 64],
                    lhsT=warm_sb[0:64, 0:80],
                    rhs=warm_sb[0:64, 128 : 128 + 64],
                    start=True,
                    stop=True,
                )

            p_tiles = {}
            o_halves = {}
            av_state = {}

            def start_av(h):
                oa = o_pool.tile([80, 512], F32, tag="ops")
                ob = o_pool.tile([80, 512], F32, tag="ops")
                av_state[h] = (oa, ob)

            def emit_av_chunk(h, half, t):
                o_ps = av_state[h][half]
                nc.tensor.matmul(
                    out=o_ps[:],
                    lhsT=v_tiles[h][:, t * 80 : (t + 1) * 80],
                    rhs=p_tiles[h][
                        :, t * S + half * 512 : t * S + half * 512 + 512
                    ],
                    start=(t == 0),
                    stop=(t == NT - 1),
                )
                if t == NT - 1:
                    ot_sb = ot_pool.tile([80, 512], BF16, name="ot_sb")
                    if h == HPC - 1:
                        # ACT is idle once its exps are done; keep the last
                        # head's casts off the busy DVE queue
                        nc.scalar.activation(
                            out=ot_sb[:],
                            in_=o_ps[:],
                            func=mybir.ActivationFunctionType.Copy,
                        )
                    else:
                        nc.vector.tensor_copy(ot_sb[:], o_ps[:])
                    o_halves.setdefault(h, []).append(ot_sb)

            def emit_tile(h, t):
                """QK -> exp -> multiplicative mask (DVE bf16 / Pool fp8)."""
                qk_sb = qk_tiles[h]
                s_ps = s_pool.tile([128, S], F32, tag="sps")
                rows = slice(0, 64) if t % 2 == 0 else slice(64, 128)
                for n in range(2):
                    sl = slice(n * 512, (n + 1) * 512)
                    nc.tensor.matmul(
                        out=s_ps[:, sl],
                        lhsT=qk_sb[rows, t * 128 : (t + 1) * 128],
                        rhs=qk_sb[rows, NT * 128 + n * 512 : NT * 128 + (n + 1) * 512],
                        start=True,
                        stop=True,
                    )
                csl = slice(t * S, (t + 1) * S)
                nc.scalar.activation(
                    out=p_tiles[h][:, csl],
                    in_=s_ps[:],
                    func=mybir.ActivationFunctionType.Exp,
                    scale=0.125,
                )
                pool_t = _pool_tiles(h)
                if t in pool_t:
                    i = pool_t.index(t)
                    nc.gpsimd.tensor_mul(
                        out=p_tiles[h][:, csl],
                        in0=p_tiles[h][:, csl],
                        in1=mpo_tiles[h][:, i * S : (i + 1) * S],
                    )
                else:
                    i = sum(1 for tt in range(t) if tt not in pool_t)
                    nc.vector.tensor_mul(
                        out=p_tiles[h][:, csl],
                        in0=p_tiles[h][:, csl],
                        in1=mdv_tiles[h][:, i * S : (i + 1) * S],
                    )

            def emit_head(h):
                """Tiles of head h with av chunks of head h-1 woven between,
                so the PE always has dependency-free work."""
                p_tiles[h] = p_pool.tile([128, NT * S], BF16, name="p_sb")
                prev = h - 1 if h >= 1 else None
                if h == 0:
                    start_av(0)
                elif h >= 2:
                    start_av(prev)
                for t in range(NT):
                    emit_tile(h, t)
                    if h == 0:
                        # self-weave two tiles behind
                        if t >= 2:
                            emit_av_chunk(0, 0, t - 2)
                            emit_av_chunk(0, 1, t - 2)
                    elif h == 1:
                        # av(0) remainder first (tiles 6,7); av(1) waits for
                        # scores(1)
                        if t == 0:
                            for tt in (6, 7):
                                emit_av_chunk(0, 0, tt)
                                emit_av_chunk(0, 1, tt)
                    else:
                        emit_av_chunk(prev, 0, t)
                        emit_av_chunk(prev, 1, t)
                    if h >= 2 and t == 4:
                        emit_tail(h - 2)

            def emit_half_tail(h, half, ot_sb):
                # one s-half: XBAR DMA transpose (off the PE) + recip + mul +
                # out DMA; f_sb[p, j, c] = ot_sb[c, j*128 + p]
                f_sb = fin_pool.tile([128, 4, 80], BF16, tag="fsb")
                nc.sync.dma_start_transpose(f_sb[:], ot_sb[:])
                r_sb = fin_pool.tile([128, 4], F32, tag="rsb")
                nc.vector.reciprocal(r_sb[:, :, None], f_sb[:, :, 64:65])
                out_sb = fin_pool.tile([128, 4 * D], BF16, tag="osb")
                nc.vector.tensor_mul(
                    out=out_sb[:].rearrange("p (j d) -> p j d", j=4),
                    in0=f_sb[:, :, 0:64],
                    in1=r_sb[:, :, None].to_broadcast((128, 4, D)),
                )
                nc.sync.dma_start(
                    outp[h][:, half * 256 : half * 256 + 256], out_sb[:]
                )

            def emit_tail(h):
                ot_sbs = o_halves.pop(h)
                emit_half_tail(h, 0, ot_sbs[0])
                emit_half_tail(h, 1, ot_sbs[1])
                p_tiles.pop(h)

            for h in range(HPC):
                emit_head(h)
            # last head: run half A's AV to completion first so its tail
            # overlaps half B's chunks
            last = HPC - 1
            start_av(last)
            for t in range(NT):
                emit_av_chunk(last, 0, t)
                if t == 2:
                    emit_tail(HPC - 2)
            for t in range(NT):
                emit_av_chunk(last, 1, t)
                if t == 1:
                    emit_half_tail(last, 0, o_halves[last][0])
            emit_half_tail(last, 1, o_halves[last][1])
            p_tiles.pop(last)
            o_halves.pop(last)

    _hoist_early_dmas(nc, max_hoist=1)
    _split_multi_waits(nc)
    return nc


def _hoist_early_dmas(nc, max_hoist=1):
    """Move the first wait-free SP input DMA(s) from the body basic block
    into the preamble block, before SP's drain. The transfer then runs during
    the ~7us of preamble engine-table loads, so the first QK's operands are
    resident when the PE enters the body. Each hoisted DMA delays the
    end-of-preamble barrier by its ~0.65us SP issue time, so only the
    critical-path qkt[0] is worth hoisting. Data safety is preserved by the
    DMA's own completion semaphore, which consumers still wait on."""
    main_bb = body_bb = None
    for name, bb in nc.bb_map.items():
        if name == "main":
            main_bb = bb.bb
        elif len(bb.bb.instructions) > 100:
            body_bb = bb.bb
    if main_bb is None or body_bb is None:
        return
    drain_idx = None
    for idx, inst in enumerate(main_bb.instructions):
        nm = type(inst).__name__
        if (
            nm in ("InstRegisterMove", "InstDrain")
            and str(getattr(inst, "engine", "")) == "EngineType.SP"
        ):
            drain_idx = idx  # first SP instruction: issue the DMA before it
            break
    if drain_idx is None:
        return
    hoist = []
    for inst in body_bb.instructions:
        if len(hoist) >= max_hoist:
            break
        if (
            type(inst).__name__ == "InstDMACopy"
            and str(getattr(inst, "engine", "")) == "EngineType.SP"
        ):
            si = getattr(inst, "sync_info", None)
            if si is not None and si.on_wait:
                break  # stop at the first dependent DMA to keep queue order
            hoist.append(inst)
    for inst in hoist:
        body_bb.instructions.remove(inst)
    main_bb.instructions[drain_idx:drain_idx] = hoist


def _split_multi_waits(nc):
    """Walrus's S3_LW codegen can't take >1 sync-wait condition on a Matmult;
    hoist extras into standalone EventSemaphore instructions (same semantics:
    the engine queue stalls on them in program order, like raw-bass wait_ge).

    Before splitting, drop subsumed waits: engine queues execute in program
    order and tile semaphores only count up, so a wait sem>=Y after an
    earlier wait sem>=X (X>=Y) on the same engine is a no-op."""
    for bb in nc.bb_map.values():
        insts = bb.bb.instructions
        seen: dict = {}
        for inst in insts:
            si = getattr(inst, "sync_info", None)
            if si is None or not si.on_wait:
                continue
            eng = getattr(inst, "engine", None)
            e_seen = seen.setdefault(eng, {})
            kept = []
            for cond in si.on_wait:
                if cond.wait_mode == "sem-ge-imm":
                    prev = e_seen.get(cond.id)
                    if prev is not None and prev >= cond.wait_value:
                        continue
                    e_seen[cond.id] = max(prev or 0, cond.wait_value)
                else:
                    # non-monotone wait: stop tracking this semaphore
                    e_seen.pop(cond.id, None)
                kept.append(cond)
            si.on_wait = kept
    for bb in nc.bb_map.values():
        insts = bb.bb.instructions
        new_list = []
        for inst in insts:
            si = getattr(inst, "sync_info", None)
            if (
                si is not None
                and si.on_wait
                and len(si.on_wait) > 1
            ):
                extra = si.on_wait[:-1]
                keep = si.on_wait[-1:]
                for cond in extra:
                    new_list.append(
                        mybir.InstEventSemaphore(
                            name=nc.get_next_instruction_name(),
                            ins=[],
                            outs=[],
                            engine=inst.engine,
                            sync_info=mybir.SyncInfo(on_wait=[cond], on_update=[]),
                        )
                    )
                si.on_wait = keep
            new_list.append(inst)
        insts[:] = new_list


import concourse.bass_utils as _bu

_orig_run_command = _bu.run_command


def _run_command_ldwopt(cmd, **kw):
    if os.environ.get("LDW_OPT") == "1":
        cmd = [
            "--enable-ldw-opt=true" if c == "--enable-ldw-opt=false" else c
            for c in cmd
        ]
    return _orig_run_command(cmd, **kw)


_bu.run_command = _run_command_ldwopt

_NC_CACHE = None


def _get_nc():
    global _NC_CACHE
    if _NC_CACHE is None:
        _NC_CACHE = _build_program()
    return _NC_CACHE


def _make_in_maps(q, k, v, mask):
    q = np.ascontiguousarray(np.asarray(q, dtype=np.float32))
    k = np.ascontiguousarray(np.asarray(k, dtype=np.float32))
    v = np.ascontiguousarray(np.asarray(v, dtype=np.float32))
    mask = np.asarray(mask)
    ones_col = np.ones((HPC, S, 1), dtype=np.float32)
    in_maps = []
    for c in range(NCORES):
        sl = slice(c * HPC, (c + 1) * HPC)
        qT = q[sl].transpose(0, 2, 1)  # [HPC, 64, S]
        kT = k[sl].transpose(0, 2, 1)
        qk1 = np.concatenate([kT, qT], axis=2)  # [HPC, 64, 2S]
        qkt_np = np.ascontiguousarray(
            np.concatenate([qk1, qk1], axis=1)
        ).astype(ml_dtypes.bfloat16)  # rows duplicated for PE quadrants
        va = np.concatenate(
            [v[sl], ones_col, np.zeros((HPC, S, 15), np.float32)], axis=2
        )  # [HPC, S, 80]: 64 dims + denominator ones + pad to 80 for XBAR
        vaug_np = np.ascontiguousarray(
            va.reshape(HPC, NT, 128, 80).transpose(0, 2, 1, 3).reshape(HPC, 128, NT * 80)
        ).astype(ml_dtypes.bfloat16)
        keepT = (~mask[sl]).transpose(0, 2, 1)  # [HPC, t, s] keep in {1,0}
        keep_tiled = (
            keepT.reshape(HPC, NT, 128, S).transpose(0, 2, 1, 3)
        )  # [HPC, 128, NT, S]
        mdv_np = np.zeros((HPC, 128, NDV * S), dtype=ml_dtypes.bfloat16)
        mpo_np = np.zeros((HPC, 128, NPO * S), dtype=FP8)
        for h in range(HPC):
            pool_t = _pool_tiles(h)
            dv = [t for t in range(NT) if t not in pool_t]
            for i, t in enumerate(dv):
                mdv_np[h, :, i * S : (i + 1) * S] = keep_tiled[h, :, t]
            for i, t in enumerate(pool_t):
                mpo_np[h, :, i * S : (i + 1) * S] = keep_tiled[h, :, t]
        in_maps.append(
            {
                "qkt": qkt_np,
                "vaug": vaug_np,
                "mdv": mdv_np,
                "mpo": mpo_np,
            }
        )
    return in_maps


def _gather(results):
    outs = []
    for c in range(NCORES):
        o = np.asarray(results[c]["outp"], dtype=np.float32)  # [HPC,128,NT*D]
        o = o.reshape(HPC, 128, NT, D).transpose(0, 2, 1, 3).reshape(HPC, S, D)
        outs.append(o)
    return np.ascontiguousarray(np.concatenate(outs, axis=0))


def _install_profile_shim():
    """The agent image's antenv lacks axon_hooks; recreate it from the boot
    module's ctypes implementation so trace=True can capture NTFF profiles."""
    import types

    if "antenv.axon_hooks" in sys.modules:
        return
    try:
        from trn_agent_boot.trn_boot import _ntff_profile_via_ctypes

        hook = _ntff_profile_via_ctypes("/opt/axon/libaxon_pjrt.so")
        mod = types.ModuleType("antenv.axon_hooks")
        mod.get_axon_ntff_profile_hook = lambda: hook
        mod.set_axon_ntff_profile_hook = lambda h: None
        sys.modules["antenv.axon_hooks"] = mod
        # don't try to copy artifacts to a remote bucket from the sandbox
        import concourse.bass_utils as _bu

        _bu.upload_artifacts = lambda tmpdir: tmpdir
    except Exception as e:  # profiling is best-effort
        print(f"profile shim unavailable: {e}", file=sys.stderr)


def run(q, k, v, mask, trace=False, **kw):
    nc = _get_nc()
    if trace:
        _install_profile_shim()
    in_maps = _make_in_maps(q, k, v, mask)
    res = run_bass_kernel_spmd(nc, in_maps, list(range(NCORES)), trace=trace, **kw)
    return _gather(res.results), res


def kernel(q, k, v, mask):
    out, _ = run(q, k, v, mask)
    return out
